# revision 48
# baseline (speedup 1.0000x reference)
"""MoE SwiGLU (top-2 of 8 experts) on 8 TRN2 NeuronCores.

Strategy: expert-parallel. The router (tiny: [N,1024]@[1024,8]) plus the
top-2 dispatch/combine permutations run on host as part of the sharding
step -- sharding by expert is only possible after routing, and the
all-to-all dispatch/combine of the sharding hint is exactly this
host-side gather/scatter under the full-I/O contract. Each core then
runs a dense SwiGLU FFN over its expert's gathered tokens (padded to a
fixed capacity C):

    yT = w3.T @ (silu(w1.T @ xT) * (w2.T @ xT))      all on-device

Everything is kept transposed ([feature, token]) so no on-device
transposes are needed: w1/w2 ([D,H]) and w3 ([H,D]) are already in lhsT
layout, x is shipped pre-transposed, y returns transposed.

Matmuls run as float32r (full fp32 data, 1 cycle/row on the PE at free
dim >= 256, vs 4 cycles/row for plain fp32). The hidden dim (2048) is
processed in two halves so the resident weight set fits SBUF; each
half's partial y is accumulated through the output DRAM buffer.

The device program is raw Bass (explicit per-engine streams and
semaphores, no TileContext): the walrus build in this container accepts
at most ONE semaphore wait per instruction, which Tile's auto-generated
sync violates structurally (slot-recycling WAR/WAW sets span multiple
procs). With explicit streams, every wait is its own single-wait
wait_ge instruction and every DMA is enqueued only after its
dependencies completed, so no instruction ever carries two waits.
"""

import numpy as np

import concourse.bass as bass
import concourse.mybir as mybir
from concourse.bass_utils import run_bass_kernel_spmd

D = 1024          # model dim
H = 2048          # expert hidden dim
HH = H // 2       # hidden half processed per pass (weight residency)
E = 8             # experts == cores
TOP_K = 2
NT = 512          # tokens per block (max fp32 moving free dim)
P = 128

F32 = mybir.dt.float32
F32R = mybir.dt.float32r

KD = D // P       # k-tiles over model dim (contraction of matmul 1/2)
TH = HH // P      # hidden tiles per half (contraction of matmul 3)
OD = D // P       # output dim tiles

_PROGRAM_CACHE: dict[int, bass.Bass] = {}
LAST_RESULTS = None  # BassKernelResults of the most recent run (for test harness)


def _build_program(C: int) -> bass.Bass:
    """One core's program: dense SwiGLU FFN over C tokens, transposed layout."""
    NB = C // NT
    NBT = 2 * NB            # total block passes (two hidden halves)
    nc = bass.Bass("TRN2", target_bir_lowering=False, debug=False)
    xT_d = nc.dram_tensor("xT", [D, C], F32R, kind="ExternalInput").ap()
    w1_d = nc.dram_tensor("w1", [D, H], F32R, kind="ExternalInput").ap()
    w2_d = nc.dram_tensor("w2", [D, H], F32R, kind="ExternalInput").ap()
    w3_d = nc.dram_tensor("w3", [H, D], F32R, kind="ExternalInput").ap()
    yT_d = nc.dram_tensor("yT", [D, C], F32, kind="ExternalOutput").ap()

    def dram3(t, c0):
        # [D, NT] column block as [p, k, n] for a single strided DMA
        return t[:, c0:c0 + NT].rearrange("(k p) n -> p k n", p=P)

    from contextlib import ExitStack
    with ExitStack() as ctx:
        sb = lambda name, cols, dt=F32R: ctx.enter_context(
            nc.sbuf_tensor(name, [P, cols], dt))
        ps = lambda name: ctx.enter_context(
            nc.psum_tensor(name, [P, NT], F32))
        w1s = sb("w1s", KD * HH)
        w2s = sb("w2s", KD * HH)
        w3s = sb("w3s", TH * D)
        xTs = [sb(f"xTs{i}", KD * NT) for i in range(2)]
        hTs = [sb(f"hTs{i}", TH * NT) for i in range(2)]
        ss = [sb(f"ss{i}", NT, F32) for i in range(2)]
        sa = [sb(f"sa{i}", NT, F32) for i in range(2)]
        yb = sb("yb", OD * NT, F32)
        yp = sb("yp", OD * NT, F32)
        pa = [ps(f"pa{i}") for i in range(2)]
        pb = [ps(f"pb{i}") for i in range(2)]
        py = [ps(f"py{i}") for i in range(2)]

        sem = lambda name: ctx.enter_context(nc.semaphore(name))
        # Weight loads are split into per-slice DMAs with one sem each:
        # value-gating a single sem across DMAs that complete out of order
        # is unsound (any 16-increment would satisfy the wait).
        s_w1s = [sem(f"s_w1s{j}") for j in range(TH)]
        s_w2s = [sem(f"s_w2s{j}") for j in range(TH)]
        s_w3s = [sem(f"s_w3s{j}") for j in range(OD)]
        s_x = [sem(f"s_x{i}") for i in range(2)]   # +16 per x DMA, by slot parity
        s_pa = sem("s_pa")        # +1 per finished pa accumulation group
        s_pb = sem("s_pb")        # +1 per finished pb accumulation group
        s_act = sem("s_act")      # +1 per sigmoid done on ACT (ss written)
        s_sa = sem("s_sa")        # +1 per sa mul done on DVE (ss + pa free)
        s_hT = sem("s_hT")        # +1 per hT tile written by DVE (pb free)
        s_py = sem("s_py")        # +1 per finished py accumulation group
        s_ydone = sem("s_ydone")  # +1 per yb column tile staged by DVE (py free)
        s_yps = [sem(f"s_yps{j}") for j in range(OD)]  # +16 per od reload
        s_store = sem("s_store")  # +16 per y store DMA

        block = ctx.enter_context(nc.Block())

        @block.sync
        def _(sync):
            for h in range(2):
                hs = h * HH
                # Weight loads, sliced so the half's first block can start
                # after one slice; w1/w2 slices interleaved to match the
                # pa/pb consumption order, x block 0 in front of everything.
                # Reloads gate per slice on the last half-0 reader of that
                # slice (PE completes in order).
                if h == 0:
                    sync.dma_start(
                        xTs[0][:].rearrange("p (k n) -> p k n", k=KD),
                        dram3(xT_d, 0),
                    ).then_inc(s_x[0], 16)
                w1r = w1s[:].rearrange("p (k c) -> p k c", k=KD)
                w2r = w2s[:].rearrange("p (k c) -> p k c", k=KD)
                for j in range(TH):
                    if h == 1:
                        sync.wait_ge(s_pa, (NB - 1) * TH + j + 1)
                    sync.dma_start(
                        w1r[:, :, j * P:(j + 1) * P],
                        w1_d[:, hs + j * P: hs + (j + 1) * P]
                        .rearrange("(k p) c -> p k c", p=P),
                    ).then_inc(s_w1s[j], 16)
                    if h == 1:
                        sync.wait_ge(s_pb, (NB - 1) * TH + j + 1)
                    sync.dma_start(
                        w2r[:, :, j * P:(j + 1) * P],
                        w2_d[:, hs + j * P: hs + (j + 1) * P]
                        .rearrange("(k p) c -> p k c", p=P),
                    ).then_inc(s_w2s[j], 16)
                    if h == 0 and j == 0:
                        sync.dma_start(
                            xTs[1][:].rearrange("p (k n) -> p k n", k=KD),
                            dram3(xT_d, (1 % NB) * NT),
                        ).then_inc(s_x[1], 16)
                w3r = w3s[:].rearrange("p (k c) -> p k c", k=TH)
                for j in range(OD):
                    if h == 1:
                        sync.wait_ge(s_py, (NB - 1) * OD + j + 1)
                    sync.dma_start(
                        w3r[:, :, j * P:(j + 1) * P],
                        w3_d[hs:hs + HH, j * P:(j + 1) * P]
                        .rearrange("(k p) c -> p k c", p=P),
                    ).then_inc(s_w3s[j], 16)
                if h == 1:
                    # all half-0 partial stores complete before first reload
                    sync.wait_ge(s_store, 16 * OD * NB)
                    for od in range(OD):
                        sync.dma_start(
                            yp[:, od * NT:(od + 1) * NT],
                            yT_d[od * P:(od + 1) * P, 0:NT],
                        ).then_inc(s_yps[od], 16)
                for b in range(NB):
                    B = h * NB + b
                    # prefetch x two blocks ahead (the store waits below
                    # resolve deep into block B+1, so one-ahead is too late):
                    # slot (B+2)%2 is free once PE's pb groups of block B
                    # completed (pb is the last x reader, in-order)
                    if B + 2 < NBT:
                        sync.wait_ge(s_pb, TH * (B + 1))
                        sync.dma_start(
                            xTs[B % 2][:]
                            .rearrange("p (k n) -> p k n", k=KD),
                            dram3(xT_d, ((B + 2) % NB) * NT),
                        ).then_inc(s_x[B % 2], 16)
                    # store each staged od tile of block B as soon as it's
                    # ready (copies of block B land during global block B+1);
                    # in half 1 the od-slice of the next y-partial reload can
                    # go out on the same gate (its slot's reader just ran)
                    for od in range(OD):
                        sync.wait_ge(s_ydone, OD * B + od + 1)
                        if h == 1 and b + 1 < NB:
                            sync.dma_start(
                                yp[:, od * NT:(od + 1) * NT],
                                yT_d[od * P:(od + 1) * P,
                                     (b + 1) * NT:(b + 2) * NT],
                            ).then_inc(s_yps[od], 16)
                        sync.dma_start(
                            yT_d[od * P:(od + 1) * P, b * NT:(b + 1) * NT],
                            yb[:, od * NT:(od + 1) * NT],
                        ).then_inc(s_store, 16)
            sync.wait_ge(s_store, 16 * OD * NBT)

        @block.tensor
        def _(tensor):
            def py_group(Bp, od):
                # third matmul for block Bp, interleaved into block Bp+1
                hp, bp = divmod(Bp, NB)
                o = Bp * OD + od
                if od == 0:
                    tensor.wait_ge(s_hT, TH * (Bp + 1))    # hT block complete
                if bp == 0:
                    tensor.wait_ge(s_w3s[od], 16 * (hp + 1))
                if o >= 2:
                    tensor.wait_ge(s_ydone, o - 1)         # py slot free
                for k in range(TH):
                    mm = tensor.matmul(
                        py[o % 2][:],
                        w3s[:, k * D + od * P: k * D + od * P + P],
                        hTs[Bp % 2][:, k * NT: (k + 1) * NT],
                        start=(k == 0), stop=(k == TH - 1),
                    )
                mm.then_inc(s_py, 1)

            for h in range(2):
                for b in range(NB):
                    B = h * NB + b
                    tensor.wait_ge(s_x[B % 2], 16 * (B // 2 + 1))
                    for ht in range(TH):
                        g = B * TH + ht
                        if b == 0:
                            tensor.wait_ge(s_w1s[ht], 16 * (h + 1))
                        if g >= 2:
                            tensor.wait_ge(s_sa, g - 1)    # pa slot free
                        for k in range(KD):
                            mm = tensor.matmul(
                                pa[g % 2][:],
                                w1s[:, k * HH + ht * P: k * HH + ht * P + P],
                                xTs[B % 2][:, k * NT: (k + 1) * NT],
                                start=(k == 0), stop=(k == KD - 1),
                            )
                        mm.then_inc(s_pa, 1)
                        if b == 0:
                            tensor.wait_ge(s_w2s[ht], 16 * (h + 1))
                        if g >= 2:
                            tensor.wait_ge(s_hT, g - 1)    # pb slot free
                        for k in range(KD):
                            mm = tensor.matmul(
                                pb[g % 2][:],
                                w2s[:, k * HH + ht * P: k * HH + ht * P + P],
                                xTs[B % 2][:, k * NT: (k + 1) * NT],
                                start=(k == 0), stop=(k == KD - 1),
                            )
                        mm.then_inc(s_pb, 1)
                        if b >= 1:
                            py_group(B - 1, ht)
                # the half's last block can't defer its py groups into the
                # next half: the weight reloads would deadlock against them
                for od in range(OD):
                    py_group(h * NB + NB - 1, od)

        @block.scalar
        def _(scalar):
            for g in range(NBT * TH):
                scalar.wait_ge(s_pa, g + 1)
                if g >= 2:
                    scalar.wait_ge(s_sa, g - 1)            # ss slot free
                scalar.activation(ss[g % 2][:], pa[g % 2][:],
                                  mybir.ActivationFunctionType.Sigmoid
                                  ).then_inc(s_act, 1)

        @block.vector
        def _(vector):
            def stage_y(Bp, od):
                # stage block Bp's od tile into yb (runs during block Bp+1)
                hp, bp = divmod(Bp, NB)
                o = Bp * OD + od
                if hp == 1:
                    vector.wait_ge(s_yps[od], 16 * (bp + 1))  # reload landed
                if od == 0 and Bp >= 1:
                    vector.wait_ge(s_store, 16 * OD * Bp)      # yb free
                vector.wait_ge(s_py, o + 1)
                if hp == 0:
                    vector.tensor_copy(
                        yb[:, od * NT: (od + 1) * NT],
                        py[o % 2][:]).then_inc(s_ydone, 1)
                else:
                    vector.tensor_add(
                        yb[:, od * NT: (od + 1) * NT], py[o % 2][:],
                        yp[:, od * NT: (od + 1) * NT],
                    ).then_inc(s_ydone, 1)

            for B in range(NBT):
                if B >= 2:
                    vector.wait_ge(s_py, OD * (B - 1))  # hTs[B%2] free
                for ht in range(TH):
                    g = B * TH + ht
                    vector.wait_ge(s_act, g + 1)
                    vector.tensor_mul(sa[g % 2][:], ss[g % 2][:],
                                      pa[g % 2][:]).then_inc(s_sa, 1)
                    vector.wait_ge(s_sa, g + 1)   # DVE completes async: RAW
                    vector.wait_ge(s_pb, g + 1)
                    vector.tensor_mul(hTs[B % 2][:, ht * NT: (ht + 1) * NT],
                                      sa[g % 2][:], pb[g % 2][:]
                                      ).then_inc(s_hT, 1)
                    if B % NB >= 1:
                        stage_y(B - 1, ht)
                if B % NB == NB - 1:
                    for od in range(OD):
                        stage_y(B, od)
    return nc


def _route(xf: np.ndarray, router_w: np.ndarray):
    """Host router: softmax top-2 with renormalized gates (fp32, matches ref)."""
    logits = xf @ router_w.T                      # [N, E]
    m = logits.max(axis=-1, keepdims=True)
    ex = np.exp(logits - m)
    probs = ex / ex.sum(axis=-1, keepdims=True)
    order = np.argsort(-probs, axis=-1, kind="stable")   # ties: lower idx first
    top2 = order[:, :TOP_K]                       # [N, 2]
    vals = np.take_along_axis(probs, top2, axis=-1)
    gates = vals / vals.sum(axis=-1, keepdims=True)
    return top2, gates.astype(np.float32)


def kernel(x, router_w, w1, w2, w3):
    x = np.asarray(x, np.float32)
    router_w = np.asarray(router_w, np.float32)
    w1 = np.ascontiguousarray(np.asarray(w1, np.float32))
    w2 = np.ascontiguousarray(np.asarray(w2, np.float32))
    w3 = np.ascontiguousarray(np.asarray(w3, np.float32))
    orig_shape = x.shape
    xf = np.ascontiguousarray(x.reshape(-1, D))
    N = xf.shape[0]

    top2, gates = _route(xf, router_w)

    # Dispatch: token index list per expert (token order preserved).
    idx = [np.where((top2 == e).any(axis=1))[0] for e in range(E)]
    gate = [
        np.where(top2[idx[e]] == e, gates[idx[e]], 0.0).sum(axis=1).astype(np.float32)
        for e in range(E)
    ]
    counts = [len(i) for i in idx]
    C = max(((max(counts) + NT - 1) // NT) * NT, 2 * NT)

    if C not in _PROGRAM_CACHE:
        _PROGRAM_CACHE[C] = _build_program(C)
    nc = _PROGRAM_CACHE[C]

    in_maps = []
    for e in range(E):
        xg = np.zeros((C, D), np.float32)
        xg[:counts[e]] = xf[idx[e]]
        in_maps.append({
            "xT": np.ascontiguousarray(xg.T),
            "w1": w1[e],
            "w2": w2[e],
            "w3": w3[e],
        })

    res = run_bass_kernel_spmd(nc, in_maps, core_ids=list(range(E)))
    global LAST_RESULTS
    LAST_RESULTS = res

    out = np.zeros((N, D), np.float32)
    for e in range(E):
        y = res.results[e]["yT"][:, :counts[e]].T          # [count, D]
        out[idx[e]] += gate[e][:, None] * y
    return out.reshape(orig_shape), np.float32(0.0)


# revision 49
# speedup vs baseline: 1.0484x; 1.0484x over previous
"""MoE SwiGLU (top-2 of 8 experts) on 8 TRN2 NeuronCores.

Strategy: expert-parallel. The router (tiny: [N,1024]@[1024,8]) plus the
top-2 dispatch/combine permutations run on host as part of the sharding
step -- sharding by expert is only possible after routing, and the
all-to-all dispatch/combine of the sharding hint is exactly this
host-side gather/scatter under the full-I/O contract. Each core then
runs a dense SwiGLU FFN over its expert's gathered tokens (padded to a
fixed capacity C):

    yT = w3.T @ (silu(w1.T @ xT) * (w2.T @ xT))      all on-device

Everything is kept transposed ([feature, token]) so no on-device
transposes are needed: w1/w2 ([D,H]) and w3 ([H,D]) are already in lhsT
layout, x is shipped pre-transposed, y returns transposed.

Matmuls run as float32r (full fp32 data, 1 cycle/row on the PE at free
dim >= 256, vs 4 cycles/row for plain fp32). The hidden dim (2048) is
processed in two halves so the resident weight set fits SBUF; each
half's partial y is accumulated through the output DRAM buffer.

The device program is raw Bass (explicit per-engine streams and
semaphores, no TileContext): the walrus build in this container accepts
at most ONE semaphore wait per instruction, which Tile's auto-generated
sync violates structurally (slot-recycling WAR/WAW sets span multiple
procs). With explicit streams, every wait is its own single-wait
wait_ge instruction and every DMA is enqueued only after its
dependencies completed, so no instruction ever carries two waits.
"""

import numpy as np

import concourse.bass as bass
import concourse.mybir as mybir
from concourse.bass_utils import run_bass_kernel_spmd

D = 1024          # model dim
H = 2048          # expert hidden dim
HH = H // 2       # hidden half processed per pass (weight residency)
E = 8             # experts == cores
TOP_K = 2
NT = 512          # tokens per block (max fp32 moving free dim)
P = 128

F32 = mybir.dt.float32
F32R = mybir.dt.float32r

KD = D // P       # k-tiles over model dim (contraction of matmul 1/2)
TH = HH // P      # hidden tiles per half (contraction of matmul 3)
OD = D // P       # output dim tiles

_PROGRAM_CACHE: dict[int, bass.Bass] = {}
LAST_RESULTS = None  # BassKernelResults of the most recent run (for test harness)


def _build_program(C: int) -> bass.Bass:
    """One core's program: dense SwiGLU FFN over C tokens, transposed layout."""
    BS = [NT] * (C // NT) + ([C % NT] if C % NT else [])   # per-block widths
    assert C % NT in (0, 256) and len(BS) >= 2
    NB = len(BS)
    NBT = 2 * NB            # total block passes (two hidden halves)
    nc = bass.Bass("TRN2", target_bir_lowering=False, debug=False)
    xT_d = nc.dram_tensor("xT", [D, C], F32R, kind="ExternalInput").ap()
    w1_d = nc.dram_tensor("w1", [D, H], F32R, kind="ExternalInput").ap()
    w2_d = nc.dram_tensor("w2", [D, H], F32R, kind="ExternalInput").ap()
    w3_d = nc.dram_tensor("w3", [H, D], F32R, kind="ExternalInput").ap()
    yT_d = nc.dram_tensor("yT", [D, C], F32, kind="ExternalOutput").ap()

    def dram3(t, c0, w=NT):
        # [D, w] column block as [p, k, n] for a single strided DMA
        return t[:, c0:c0 + w].rearrange("(k p) n -> p k n", p=P)

    from contextlib import ExitStack
    with ExitStack() as ctx:
        sb = lambda name, cols, dt=F32R: ctx.enter_context(
            nc.sbuf_tensor(name, [P, cols], dt))
        ps = lambda name: ctx.enter_context(
            nc.psum_tensor(name, [P, NT], F32))
        w1s = sb("w1s", KD * HH)
        w2s = sb("w2s", KD * HH)
        w3s = sb("w3s", TH * D)
        xTs = [sb(f"xTs{i}", KD * NT) for i in range(2)]
        hTs = [sb(f"hTs{i}", TH * NT) for i in range(2)]
        ss = [sb(f"ss{i}", NT, F32) for i in range(2)]
        sa = [sb(f"sa{i}", NT, F32) for i in range(2)]
        yb = sb("yb", OD * NT, F32)
        yp = sb("yp", OD * NT, F32)
        pa = [ps(f"pa{i}") for i in range(2)]
        pb = [ps(f"pb{i}") for i in range(2)]
        py = [ps(f"py{i}") for i in range(2)]

        sem = lambda name: ctx.enter_context(nc.semaphore(name))
        # Weight loads are split into per-slice DMAs with one sem each:
        # value-gating a single sem across DMAs that complete out of order
        # is unsound (any 16-increment would satisfy the wait).
        s_w1s = [sem(f"s_w1s{j}") for j in range(TH)]
        s_w2s = [sem(f"s_w2s{j}") for j in range(TH)]
        s_w3s = [sem(f"s_w3s{j}") for j in range(OD)]
        s_x = [sem(f"s_x{i}") for i in range(2)]   # +16 per x DMA, by slot parity
        s_pa = sem("s_pa")        # +1 per finished pa accumulation group
        s_pb = sem("s_pb")        # +1 per finished pb accumulation group
        s_act = sem("s_act")      # +1 per sigmoid done on ACT (ss written)
        s_sa = sem("s_sa")        # +1 per sa mul done on DVE (ss + pa free)
        s_hT = sem("s_hT")        # +1 per hT tile written by DVE (pb free)
        s_py = sem("s_py")        # +1 per finished py accumulation group
        s_ydone = sem("s_ydone")  # +1 per yb column tile staged by DVE (py free)
        s_yps = [sem(f"s_yps{j}") for j in range(OD)]  # +16 per od reload
        s_store = sem("s_store")  # +16 per y store DMA

        block = ctx.enter_context(nc.Block())

        @block.sync
        def _(sync):
            for h in range(2):
                hs = h * HH
                # Weight loads, sliced so the half's first block can start
                # after one slice; w1/w2 slices interleaved to match the
                # pa/pb consumption order, x block 0 in front of everything.
                # Reloads gate per slice on the last half-0 reader of that
                # slice (PE completes in order).
                if h == 0:
                    sync.dma_start(
                        xTs[0][:].rearrange("p (k n) -> p k n", k=KD),
                        dram3(xT_d, 0),
                    ).then_inc(s_x[0], 16)
                w1r = w1s[:].rearrange("p (k c) -> p k c", k=KD)
                w2r = w2s[:].rearrange("p (k c) -> p k c", k=KD)
                for j in range(TH):
                    if h == 1:
                        sync.wait_ge(s_pa, (NB - 1) * TH + j + 1)
                    sync.dma_start(
                        w1r[:, :, j * P:(j + 1) * P],
                        w1_d[:, hs + j * P: hs + (j + 1) * P]
                        .rearrange("(k p) c -> p k c", p=P),
                    ).then_inc(s_w1s[j], 16)
                    if h == 1:
                        sync.wait_ge(s_pb, (NB - 1) * TH + j + 1)
                    sync.dma_start(
                        w2r[:, :, j * P:(j + 1) * P],
                        w2_d[:, hs + j * P: hs + (j + 1) * P]
                        .rearrange("(k p) c -> p k c", p=P),
                    ).then_inc(s_w2s[j], 16)
                    if h == 0 and j == 0:
                        sync.dma_start(
                            xTs[1][:].rearrange("p (k n) -> p k n", k=KD),
                            dram3(xT_d, (1 % NB) * NT),
                        ).then_inc(s_x[1], 16)
                w3r = w3s[:].rearrange("p (k c) -> p k c", k=TH)
                for j in range(OD):
                    if h == 1:
                        sync.wait_ge(s_py, (NB - 1) * OD + j + 1)
                    sync.dma_start(
                        w3r[:, :, j * P:(j + 1) * P],
                        w3_d[hs:hs + HH, j * P:(j + 1) * P]
                        .rearrange("(k p) c -> p k c", p=P),
                    ).then_inc(s_w3s[j], 16)
                if h == 1:
                    # all half-0 partial stores complete before first reload
                    sync.wait_ge(s_store, 16 * OD * NB)
                    for od in range(OD):
                        sync.dma_start(
                            yp[:, od * NT: od * NT + BS[0]],
                            yT_d[od * P:(od + 1) * P, 0:BS[0]],
                        ).then_inc(s_yps[od], 16)
                for b in range(NB):
                    B = h * NB + b
                    # prefetch x two blocks ahead (the store waits below
                    # resolve deep into block B+1, so one-ahead is too late):
                    # slot (B+2)%2 is free once PE's pb groups of block B
                    # completed (pb is the last x reader, in-order)
                    if B + 2 < NBT:
                        j2 = (B + 2) % NB
                        sync.wait_ge(s_pb, TH * (B + 1))
                        sync.dma_start(
                            xTs[B % 2][:]
                            .rearrange("p (k n) -> p k n", k=KD)[:, :, :BS[j2]],
                            dram3(xT_d, j2 * NT, BS[j2]),
                        ).then_inc(s_x[B % 2], 16)
                    # store each staged od tile of block B as soon as it's
                    # ready (copies of block B land during global block B+1);
                    # in half 1 the od-slice of the next y-partial reload can
                    # go out on the same gate (its slot's reader just ran)
                    for od in range(OD):
                        sync.wait_ge(s_ydone, OD * B + od + 1)
                        if h == 1 and b + 1 < NB:
                            sync.dma_start(
                                yp[:, od * NT: od * NT + BS[b + 1]],
                                yT_d[od * P:(od + 1) * P,
                                     (b + 1) * NT: (b + 1) * NT + BS[b + 1]],
                            ).then_inc(s_yps[od], 16)
                        sync.dma_start(
                            yT_d[od * P:(od + 1) * P, b * NT: b * NT + BS[b]],
                            yb[:, od * NT: od * NT + BS[b]],
                        ).then_inc(s_store, 16)
            sync.wait_ge(s_store, 16 * OD * NBT)

        @block.tensor
        def _(tensor):
            def py_group(Bp, od):
                # third matmul for block Bp, interleaved into block Bp+1
                hp, bp = divmod(Bp, NB)
                o = Bp * OD + od
                if od == 0:
                    tensor.wait_ge(s_hT, TH * (Bp + 1))    # hT block complete
                if bp == 0:
                    tensor.wait_ge(s_w3s[od], 16 * (hp + 1))
                if o >= 2:
                    tensor.wait_ge(s_ydone, o - 1)         # py slot free
                wp = BS[Bp % NB]
                for k in range(TH):
                    mm = tensor.matmul(
                        py[o % 2][:, :wp],
                        w3s[:, k * D + od * P: k * D + od * P + P],
                        hTs[Bp % 2][:, k * NT: k * NT + wp],
                        start=(k == 0), stop=(k == TH - 1),
                    )
                mm.then_inc(s_py, 1)

            for h in range(2):
                for b in range(NB):
                    B = h * NB + b
                    tensor.wait_ge(s_x[B % 2], 16 * (B // 2 + 1))
                    for ht in range(TH):
                        g = B * TH + ht
                        if b == 0:
                            tensor.wait_ge(s_w1s[ht], 16 * (h + 1))
                        if g >= 2:
                            tensor.wait_ge(s_sa, g - 1)    # pa slot free
                        for k in range(KD):
                            mm = tensor.matmul(
                                pa[g % 2][:, :BS[b]],
                                w1s[:, k * HH + ht * P: k * HH + ht * P + P],
                                xTs[B % 2][:, k * NT: k * NT + BS[b]],
                                start=(k == 0), stop=(k == KD - 1),
                            )
                        mm.then_inc(s_pa, 1)
                        if b == 0:
                            tensor.wait_ge(s_w2s[ht], 16 * (h + 1))
                        if g >= 2:
                            tensor.wait_ge(s_hT, g - 1)    # pb slot free
                        for k in range(KD):
                            mm = tensor.matmul(
                                pb[g % 2][:, :BS[b]],
                                w2s[:, k * HH + ht * P: k * HH + ht * P + P],
                                xTs[B % 2][:, k * NT: k * NT + BS[b]],
                                start=(k == 0), stop=(k == KD - 1),
                            )
                        mm.then_inc(s_pb, 1)
                        if b >= 1:
                            py_group(B - 1, ht)
                # the half's last block can't defer its py groups into the
                # next half: the weight reloads would deadlock against them
                for od in range(OD):
                    py_group(h * NB + NB - 1, od)

        @block.scalar
        def _(scalar):
            for g in range(NBT * TH):
                w = BS[(g // TH) % NB]
                scalar.wait_ge(s_pa, g + 1)
                if g >= 2:
                    scalar.wait_ge(s_sa, g - 1)            # ss slot free
                scalar.activation(ss[g % 2][:, :w], pa[g % 2][:, :w],
                                  mybir.ActivationFunctionType.Sigmoid
                                  ).then_inc(s_act, 1)

        @block.vector
        def _(vector):
            def stage_y(Bp, od):
                # stage block Bp's od tile into yb (runs during block Bp+1)
                hp, bp = divmod(Bp, NB)
                o = Bp * OD + od
                if hp == 1:
                    vector.wait_ge(s_yps[od], 16 * (bp + 1))  # reload landed
                if od == 0 and Bp >= 1:
                    vector.wait_ge(s_store, 16 * OD * Bp)      # yb free
                wp = BS[bp]
                vector.wait_ge(s_py, o + 1)
                if hp == 0:
                    vector.tensor_copy(
                        yb[:, od * NT: od * NT + wp],
                        py[o % 2][:, :wp]).then_inc(s_ydone, 1)
                else:
                    vector.tensor_add(
                        yb[:, od * NT: od * NT + wp], py[o % 2][:, :wp],
                        yp[:, od * NT: od * NT + wp],
                    ).then_inc(s_ydone, 1)

            for B in range(NBT):
                if B >= 2:
                    vector.wait_ge(s_py, OD * (B - 1))  # hTs[B%2] free
                w = BS[B % NB]
                for ht in range(TH):
                    g = B * TH + ht
                    vector.wait_ge(s_act, g + 1)
                    vector.tensor_mul(sa[g % 2][:, :w], ss[g % 2][:, :w],
                                      pa[g % 2][:, :w]).then_inc(s_sa, 1)
                    vector.wait_ge(s_sa, g + 1)   # DVE completes async: RAW
                    vector.wait_ge(s_pb, g + 1)
                    vector.tensor_mul(hTs[B % 2][:, ht * NT: ht * NT + w],
                                      sa[g % 2][:, :w], pb[g % 2][:, :w]
                                      ).then_inc(s_hT, 1)
                    if B % NB >= 1:
                        stage_y(B - 1, ht)
                if B % NB == NB - 1:
                    for od in range(OD):
                        stage_y(B, od)
    return nc


def _route(xf: np.ndarray, router_w: np.ndarray):
    """Host router: softmax top-2 with renormalized gates (fp32, matches ref)."""
    logits = xf @ router_w.T                      # [N, E]
    m = logits.max(axis=-1, keepdims=True)
    ex = np.exp(logits - m)
    probs = ex / ex.sum(axis=-1, keepdims=True)
    order = np.argsort(-probs, axis=-1, kind="stable")   # ties: lower idx first
    top2 = order[:, :TOP_K]                       # [N, 2]
    vals = np.take_along_axis(probs, top2, axis=-1)
    gates = vals / vals.sum(axis=-1, keepdims=True)
    return top2, gates.astype(np.float32)


def kernel(x, router_w, w1, w2, w3):
    x = np.asarray(x, np.float32)
    router_w = np.asarray(router_w, np.float32)
    w1 = np.ascontiguousarray(np.asarray(w1, np.float32))
    w2 = np.ascontiguousarray(np.asarray(w2, np.float32))
    w3 = np.ascontiguousarray(np.asarray(w3, np.float32))
    orig_shape = x.shape
    xf = np.ascontiguousarray(x.reshape(-1, D))
    N = xf.shape[0]

    top2, gates = _route(xf, router_w)

    # Dispatch: token index list per expert (token order preserved).
    idx = [np.where((top2 == e).any(axis=1))[0] for e in range(E)]
    gate = [
        np.where(top2[idx[e]] == e, gates[idx[e]], 0.0).sum(axis=1).astype(np.float32)
        for e in range(E)
    ]
    counts = [len(i) for i in idx]
    C = max(((max(counts) + 255) // 256) * 256, 2 * NT)

    if C not in _PROGRAM_CACHE:
        _PROGRAM_CACHE[C] = _build_program(C)
    nc = _PROGRAM_CACHE[C]

    in_maps = []
    for e in range(E):
        xg = np.zeros((C, D), np.float32)
        xg[:counts[e]] = xf[idx[e]]
        in_maps.append({
            "xT": np.ascontiguousarray(xg.T),
            "w1": w1[e],
            "w2": w2[e],
            "w3": w3[e],
        })

    res = run_bass_kernel_spmd(nc, in_maps, core_ids=list(range(E)))
    global LAST_RESULTS
    LAST_RESULTS = res

    out = np.zeros((N, D), np.float32)
    for e in range(E):
        y = res.results[e]["yT"][:, :counts[e]].T          # [count, D]
        out[idx[e]] += gate[e][:, None] * y
    return out.reshape(orig_shape), np.float32(0.0)


# revision 50
# speedup vs baseline: 1.0484x; 1.0000x over previous
"""MoE SwiGLU (top-2 of 8 experts) on 8 TRN2 NeuronCores.

Strategy: expert-parallel. The router (tiny: [N,1024]@[1024,8]) plus the
top-2 dispatch/combine permutations run on host as part of the sharding
step -- sharding by expert is only possible after routing, and the
all-to-all dispatch/combine of the sharding hint is exactly this
host-side gather/scatter under the full-I/O contract. Each core then
runs a dense SwiGLU FFN over its expert's gathered tokens (padded to a
fixed capacity C):

    yT = w3.T @ (silu(w1.T @ xT) * (w2.T @ xT))      all on-device

Everything is kept transposed ([feature, token]) so no on-device
transposes are needed: w1/w2 ([D,H]) and w3 ([H,D]) are already in lhsT
layout, x is shipped pre-transposed, y returns transposed.

Matmuls run as float32r (full fp32 data, 1 cycle/row on the PE at free
dim >= 256, vs 4 cycles/row for plain fp32). The hidden dim (2048) is
processed in two halves so the resident weight set fits SBUF; each
half's partial y is accumulated through the output DRAM buffer.

The device program is raw Bass (explicit per-engine streams and
semaphores, no TileContext): the walrus build in this container accepts
at most ONE semaphore wait per instruction, which Tile's auto-generated
sync violates structurally (slot-recycling WAR/WAW sets span multiple
procs). With explicit streams, every wait is its own single-wait
wait_ge instruction and every DMA is enqueued only after its
dependencies completed, so no instruction ever carries two waits.
"""

import numpy as np

import concourse.bass as bass
import concourse.mybir as mybir
from concourse.bass_utils import run_bass_kernel_spmd

D = 1024          # model dim
H = 2048          # expert hidden dim
HH = H // 2       # hidden half processed per pass (weight residency)
E = 8             # experts == cores
TOP_K = 2
NT = 512          # tokens per block (max fp32 moving free dim)
P = 128

F32 = mybir.dt.float32
F32R = mybir.dt.float32r

KD = D // P       # k-tiles over model dim (contraction of matmul 1/2)
TH = HH // P      # hidden tiles per half (contraction of matmul 3)
OD = D // P       # output dim tiles

_PROGRAM_CACHE: dict[int, bass.Bass] = {}
LAST_RESULTS = None  # BassKernelResults of the most recent run (for test harness)


def _build_program(C: int) -> bass.Bass:
    """One core's program: dense SwiGLU FFN over C tokens, transposed layout."""
    BS = [NT] * (C // NT) + ([C % NT] if C % NT else [])   # per-block widths
    assert C % NT in (0, 256) and len(BS) >= 2
    NB = len(BS)
    NBT = 2 * NB            # total block passes (two hidden halves)
    nc = bass.Bass("TRN2", target_bir_lowering=False, debug=False)
    xT_d = nc.dram_tensor("xT", [D, C], F32R, kind="ExternalInput").ap()
    w1_d = nc.dram_tensor("w1", [D, H], F32R, kind="ExternalInput").ap()
    w2_d = nc.dram_tensor("w2", [D, H], F32R, kind="ExternalInput").ap()
    w3_d = nc.dram_tensor("w3", [H, D], F32R, kind="ExternalInput").ap()
    yT_d = nc.dram_tensor("yT", [D, C], F32, kind="ExternalOutput").ap()

    def dram3(t, c0, w=NT):
        # [D, w] column block as [p, k, n] for a single strided DMA
        return t[:, c0:c0 + w].rearrange("(k p) n -> p k n", p=P)

    from contextlib import ExitStack
    with ExitStack() as ctx:
        sb = lambda name, cols, dt=F32R: ctx.enter_context(
            nc.sbuf_tensor(name, [P, cols], dt))
        ps = lambda name: ctx.enter_context(
            nc.psum_tensor(name, [P, NT], F32))
        w1s = sb("w1s", KD * HH)
        w2s = sb("w2s", KD * HH)
        w3s = sb("w3s", TH * D)
        xTs = [sb(f"xTs{i}", KD * NT) for i in range(2)]
        hTs = [sb(f"hTs{i}", TH * NT) for i in range(2)]
        ss = [sb(f"ss{i}", NT, F32) for i in range(2)]
        sa = [sb(f"sa{i}", NT, F32) for i in range(2)]
        yb = sb("yb", OD * NT, F32)
        yp = sb("yp", OD * NT, F32)
        pa = [ps(f"pa{i}") for i in range(2)]
        pb = [ps(f"pb{i}") for i in range(2)]
        py = [ps(f"py{i}") for i in range(2)]

        sem = lambda name: ctx.enter_context(nc.semaphore(name))
        # Weight loads are split into per-slice DMAs with one sem each:
        # value-gating a single sem across DMAs that complete out of order
        # is unsound (any 16-increment would satisfy the wait).
        s_w1s = [sem(f"s_w1s{j}") for j in range(TH)]
        s_w2s = [sem(f"s_w2s{j}") for j in range(TH)]
        s_w3s = [sem(f"s_w3s{j}") for j in range(OD)]
        s_x = [sem(f"s_x{i}") for i in range(2)]   # +16 per x DMA, by slot parity
        s_x0k = [sem(f"s_x0k{k}") for k in range(KD)]  # +16 per k-tile of x(0)
        s_pa = sem("s_pa")        # +1 per finished pa accumulation group
        s_pb = sem("s_pb")        # +1 per finished pb accumulation group
        s_act = sem("s_act")      # +1 per sigmoid done on ACT (ss written)
        s_sa = sem("s_sa")        # +1 per sa mul done on DVE (ss + pa free)
        s_hT = sem("s_hT")        # +1 per hT tile written by DVE (pb free)
        s_py = sem("s_py")        # +1 per finished py accumulation group
        s_ydone = sem("s_ydone")  # +1 per yb column tile staged by DVE (py free)
        s_yps = [sem(f"s_yps{j}") for j in range(OD)]  # +16 per od reload
        s_store = sem("s_store")  # +16 per y store DMA

        block = ctx.enter_context(nc.Block())

        @block.sync
        def _(sync):
            for h in range(2):
                hs = h * HH
                # Weight loads, sliced so the half's first block can start
                # after one slice; w1/w2 slices interleaved to match the
                # pa/pb consumption order, x block 0 in front of everything.
                # Reloads gate per slice on the last half-0 reader of that
                # slice (PE completes in order).
                if h == 0:
                    # x block 0 split per k-tile so the very first pa group
                    # starts after one 256KB piece instead of the whole 2MB
                    for k in range(KD):
                        sync.dma_start(
                            xTs[0][:, k * NT: k * NT + BS[0]],
                            xT_d[k * P:(k + 1) * P, 0:BS[0]],
                        ).then_inc(s_x0k[k], 16)
                w1r = w1s[:].rearrange("p (k c) -> p k c", k=KD)
                w2r = w2s[:].rearrange("p (k c) -> p k c", k=KD)
                for j in range(TH):
                    if h == 1:
                        sync.wait_ge(s_pa, (NB - 1) * TH + j + 1)
                    sync.dma_start(
                        w1r[:, :, j * P:(j + 1) * P],
                        w1_d[:, hs + j * P: hs + (j + 1) * P]
                        .rearrange("(k p) c -> p k c", p=P),
                    ).then_inc(s_w1s[j], 16)
                    if h == 1:
                        sync.wait_ge(s_pb, (NB - 1) * TH + j + 1)
                    sync.dma_start(
                        w2r[:, :, j * P:(j + 1) * P],
                        w2_d[:, hs + j * P: hs + (j + 1) * P]
                        .rearrange("(k p) c -> p k c", p=P),
                    ).then_inc(s_w2s[j], 16)
                    if h == 0 and j == 0:
                        sync.dma_start(
                            xTs[1][:].rearrange("p (k n) -> p k n", k=KD),
                            dram3(xT_d, (1 % NB) * NT),
                        ).then_inc(s_x[1], 16)
                w3r = w3s[:].rearrange("p (k c) -> p k c", k=TH)
                for j in range(OD):
                    if h == 1:
                        sync.wait_ge(s_py, (NB - 1) * OD + j + 1)
                    sync.dma_start(
                        w3r[:, :, j * P:(j + 1) * P],
                        w3_d[hs:hs + HH, j * P:(j + 1) * P]
                        .rearrange("(k p) c -> p k c", p=P),
                    ).then_inc(s_w3s[j], 16)
                if h == 1:
                    # all half-0 partial stores complete before first reload
                    sync.wait_ge(s_store, 16 * OD * NB)
                    for od in range(OD):
                        sync.dma_start(
                            yp[:, od * NT: od * NT + BS[0]],
                            yT_d[od * P:(od + 1) * P, 0:BS[0]],
                        ).then_inc(s_yps[od], 16)
                for b in range(NB):
                    B = h * NB + b
                    # prefetch x two blocks ahead (the store waits below
                    # resolve deep into block B+1, so one-ahead is too late):
                    # slot (B+2)%2 is free once PE's pb groups of block B
                    # completed (pb is the last x reader, in-order)
                    if B + 2 < NBT:
                        j2 = (B + 2) % NB
                        sync.wait_ge(s_pb, TH * (B + 1))
                        sync.dma_start(
                            xTs[B % 2][:]
                            .rearrange("p (k n) -> p k n", k=KD)[:, :, :BS[j2]],
                            dram3(xT_d, j2 * NT, BS[j2]),
                        ).then_inc(s_x[B % 2], 16)
                    # store each staged od tile of block B as soon as it's
                    # ready (copies of block B land during global block B+1);
                    # in half 1 the od-slice of the next y-partial reload can
                    # go out on the same gate (its slot's reader just ran)
                    for od in range(OD):
                        sync.wait_ge(s_ydone, OD * B + od + 1)
                        if h == 1 and b + 1 < NB:
                            sync.dma_start(
                                yp[:, od * NT: od * NT + BS[b + 1]],
                                yT_d[od * P:(od + 1) * P,
                                     (b + 1) * NT: (b + 1) * NT + BS[b + 1]],
                            ).then_inc(s_yps[od], 16)
                        sync.dma_start(
                            yT_d[od * P:(od + 1) * P, b * NT: b * NT + BS[b]],
                            yb[:, od * NT: od * NT + BS[b]],
                        ).then_inc(s_store, 16)
            sync.wait_ge(s_store, 16 * OD * NBT)

        @block.tensor
        def _(tensor):
            def py_group(Bp, od):
                # third matmul for block Bp, interleaved into block Bp+1
                hp, bp = divmod(Bp, NB)
                o = Bp * OD + od
                if od == 0:
                    tensor.wait_ge(s_hT, TH * (Bp + 1))    # hT block complete
                if bp == 0:
                    tensor.wait_ge(s_w3s[od], 16 * (hp + 1))
                if o >= 2:
                    tensor.wait_ge(s_ydone, o - 1)         # py slot free
                wp = BS[Bp % NB]
                for k in range(TH):
                    mm = tensor.matmul(
                        py[o % 2][:, :wp],
                        w3s[:, k * D + od * P: k * D + od * P + P],
                        hTs[Bp % 2][:, k * NT: k * NT + wp],
                        start=(k == 0), stop=(k == TH - 1),
                    )
                mm.then_inc(s_py, 1)

            for h in range(2):
                for b in range(NB):
                    B = h * NB + b
                    if B == 0:
                        pass          # gated per k-tile inside the first group
                    elif B % 2 == 0:
                        # the x(0) preamble no longer bumps s_x[0]: even
                        # blocks' counts shift down by one
                        tensor.wait_ge(s_x[0], 16 * (B // 2))
                    else:
                        tensor.wait_ge(s_x[1], 16 * (B // 2 + 1))
                    for ht in range(TH):
                        g = B * TH + ht
                        if b == 0:
                            tensor.wait_ge(s_w1s[ht], 16 * (h + 1))
                        if g >= 2:
                            tensor.wait_ge(s_sa, g - 1)    # pa slot free
                        for k in range(KD):
                            if B == 0 and ht == 0:
                                tensor.wait_ge(s_x0k[k], 16)
                            mm = tensor.matmul(
                                pa[g % 2][:, :BS[b]],
                                w1s[:, k * HH + ht * P: k * HH + ht * P + P],
                                xTs[B % 2][:, k * NT: k * NT + BS[b]],
                                start=(k == 0), stop=(k == KD - 1),
                            )
                        mm.then_inc(s_pa, 1)
                        if b == 0:
                            tensor.wait_ge(s_w2s[ht], 16 * (h + 1))
                        if g >= 2:
                            tensor.wait_ge(s_hT, g - 1)    # pb slot free
                        for k in range(KD):
                            mm = tensor.matmul(
                                pb[g % 2][:, :BS[b]],
                                w2s[:, k * HH + ht * P: k * HH + ht * P + P],
                                xTs[B % 2][:, k * NT: k * NT + BS[b]],
                                start=(k == 0), stop=(k == KD - 1),
                            )
                        mm.then_inc(s_pb, 1)
                        if b >= 1:
                            py_group(B - 1, ht)
                # the half's last block can't defer its py groups into the
                # next half: the weight reloads would deadlock against them
                for od in range(OD):
                    py_group(h * NB + NB - 1, od)

        @block.scalar
        def _(scalar):
            for g in range(NBT * TH):
                w = BS[(g // TH) % NB]
                scalar.wait_ge(s_pa, g + 1)
                if g >= 2:
                    scalar.wait_ge(s_sa, g - 1)            # ss slot free
                scalar.activation(ss[g % 2][:, :w], pa[g % 2][:, :w],
                                  mybir.ActivationFunctionType.Sigmoid
                                  ).then_inc(s_act, 1)

        @block.vector
        def _(vector):
            def stage_y(Bp, od):
                # stage block Bp's od tile into yb (runs during block Bp+1)
                hp, bp = divmod(Bp, NB)
                o = Bp * OD + od
                if hp == 1:
                    vector.wait_ge(s_yps[od], 16 * (bp + 1))  # reload landed
                if od == 0 and Bp >= 1:
                    vector.wait_ge(s_store, 16 * OD * Bp)      # yb free
                wp = BS[bp]
                vector.wait_ge(s_py, o + 1)
                if hp == 0:
                    vector.tensor_copy(
                        yb[:, od * NT: od * NT + wp],
                        py[o % 2][:, :wp]).then_inc(s_ydone, 1)
                else:
                    vector.tensor_add(
                        yb[:, od * NT: od * NT + wp], py[o % 2][:, :wp],
                        yp[:, od * NT: od * NT + wp],
                    ).then_inc(s_ydone, 1)

            for B in range(NBT):
                if B >= 2:
                    vector.wait_ge(s_py, OD * (B - 1))  # hTs[B%2] free
                w = BS[B % NB]
                for ht in range(TH):
                    g = B * TH + ht
                    vector.wait_ge(s_act, g + 1)
                    vector.tensor_mul(sa[g % 2][:, :w], ss[g % 2][:, :w],
                                      pa[g % 2][:, :w]).then_inc(s_sa, 1)
                    vector.wait_ge(s_sa, g + 1)   # DVE completes async: RAW
                    vector.wait_ge(s_pb, g + 1)
                    vector.tensor_mul(hTs[B % 2][:, ht * NT: ht * NT + w],
                                      sa[g % 2][:, :w], pb[g % 2][:, :w]
                                      ).then_inc(s_hT, 1)
                    if B % NB >= 1:
                        stage_y(B - 1, ht)
                if B % NB == NB - 1:
                    for od in range(OD):
                        stage_y(B, od)
    return nc


def _route(xf: np.ndarray, router_w: np.ndarray):
    """Host router: softmax top-2 with renormalized gates (fp32, matches ref)."""
    logits = xf @ router_w.T                      # [N, E]
    m = logits.max(axis=-1, keepdims=True)
    ex = np.exp(logits - m)
    probs = ex / ex.sum(axis=-1, keepdims=True)
    order = np.argsort(-probs, axis=-1, kind="stable")   # ties: lower idx first
    top2 = order[:, :TOP_K]                       # [N, 2]
    vals = np.take_along_axis(probs, top2, axis=-1)
    gates = vals / vals.sum(axis=-1, keepdims=True)
    return top2, gates.astype(np.float32)


def kernel(x, router_w, w1, w2, w3):
    x = np.asarray(x, np.float32)
    router_w = np.asarray(router_w, np.float32)
    w1 = np.ascontiguousarray(np.asarray(w1, np.float32))
    w2 = np.ascontiguousarray(np.asarray(w2, np.float32))
    w3 = np.ascontiguousarray(np.asarray(w3, np.float32))
    orig_shape = x.shape
    xf = np.ascontiguousarray(x.reshape(-1, D))
    N = xf.shape[0]

    top2, gates = _route(xf, router_w)

    # Dispatch: token index list per expert (token order preserved).
    idx = [np.where((top2 == e).any(axis=1))[0] for e in range(E)]
    gate = [
        np.where(top2[idx[e]] == e, gates[idx[e]], 0.0).sum(axis=1).astype(np.float32)
        for e in range(E)
    ]
    counts = [len(i) for i in idx]
    C = max(((max(counts) + 255) // 256) * 256, 2 * NT)

    if C not in _PROGRAM_CACHE:
        _PROGRAM_CACHE[C] = _build_program(C)
    nc = _PROGRAM_CACHE[C]

    in_maps = []
    for e in range(E):
        xg = np.zeros((C, D), np.float32)
        xg[:counts[e]] = xf[idx[e]]
        in_maps.append({
            "xT": np.ascontiguousarray(xg.T),
            "w1": w1[e],
            "w2": w2[e],
            "w3": w3[e],
        })

    res = run_bass_kernel_spmd(nc, in_maps, core_ids=list(range(E)))
    global LAST_RESULTS
    LAST_RESULTS = res

    out = np.zeros((N, D), np.float32)
    for e in range(E):
        y = res.results[e]["yT"][:, :counts[e]].T          # [count, D]
        out[idx[e]] += gate[e][:, None] * y
    return out.reshape(orig_shape), np.float32(0.0)


# revision 52
# speedup vs baseline: 1.0603x; 1.0114x over previous
"""MoE SwiGLU (top-2 of 8 experts) on 8 TRN2 NeuronCores.

Strategy: expert-parallel. The router (tiny: [N,1024]@[1024,8]) plus the
top-2 dispatch/combine permutations run on host as part of the sharding
step -- sharding by expert is only possible after routing, and the
all-to-all dispatch/combine of the sharding hint is exactly this
host-side gather/scatter under the full-I/O contract. Each core then
runs a dense SwiGLU FFN over its expert's gathered tokens (padded to a
fixed capacity C):

    yT = w3.T @ (silu(w1.T @ xT) * (w2.T @ xT))      all on-device

Everything is kept transposed ([feature, token]) so no on-device
transposes are needed: w1/w2 ([D,H]) and w3 ([H,D]) are already in lhsT
layout, x is shipped pre-transposed, y returns transposed.

Matmuls run as float32r (full fp32 data, 1 cycle/row on the PE at free
dim >= 256, vs 4 cycles/row for plain fp32). The hidden dim (2048) is
processed in two halves so the resident weight set fits SBUF; each
half's partial y is accumulated through the output DRAM buffer.

The device program is raw Bass (explicit per-engine streams and
semaphores, no TileContext): the walrus build in this container accepts
at most ONE semaphore wait per instruction, which Tile's auto-generated
sync violates structurally (slot-recycling WAR/WAW sets span multiple
procs). With explicit streams, every wait is its own single-wait
wait_ge instruction and every DMA is enqueued only after its
dependencies completed, so no instruction ever carries two waits.
"""

import numpy as np

import concourse.bass as bass
import concourse.mybir as mybir
from concourse.bass_utils import run_bass_kernel_spmd

D = 1024          # model dim
H = 2048          # expert hidden dim
HH = H // 2       # hidden half processed per pass (weight residency)
E = 8             # experts == cores
TOP_K = 2
NT = 512          # tokens per block (max fp32 moving free dim)
P = 128

F32 = mybir.dt.float32
F32R = mybir.dt.float32r

KD = D // P       # k-tiles over model dim (contraction of matmul 1/2)
TH = HH // P      # hidden tiles per half (contraction of matmul 3)
OD = D // P       # output dim tiles

_PROGRAM_CACHE: dict[int, bass.Bass] = {}
LAST_RESULTS = None  # BassKernelResults of the most recent run (for test harness)


def _build_program(C: int) -> bass.Bass:
    """One core's program: dense SwiGLU FFN over C tokens, transposed layout."""
    BS = [NT] * (C // NT) + ([C % NT] if C % NT else [])   # per-block widths
    assert C % NT in (0, 256) and len(BS) >= 2
    NB = len(BS)
    NBT = 2 * NB            # total block passes (two hidden halves)
    nc = bass.Bass("TRN2", target_bir_lowering=False, debug=False)
    xT_d = nc.dram_tensor("xT", [D, C], F32R, kind="ExternalInput").ap()
    w1_d = nc.dram_tensor("w1", [D, H], F32R, kind="ExternalInput").ap()
    w2_d = nc.dram_tensor("w2", [D, H], F32R, kind="ExternalInput").ap()
    w3_d = nc.dram_tensor("w3", [H, D], F32R, kind="ExternalInput").ap()
    yT_d = nc.dram_tensor("yT", [D, C], F32, kind="ExternalOutput").ap()

    def dram3(t, c0, w=NT):
        # [D, w] column block as [p, k, n] for a single strided DMA
        return t[:, c0:c0 + w].rearrange("(k p) n -> p k n", p=P)

    from contextlib import ExitStack
    with ExitStack() as ctx:
        sb = lambda name, cols, dt=F32R: ctx.enter_context(
            nc.sbuf_tensor(name, [P, cols], dt))
        ps = lambda name: ctx.enter_context(
            nc.psum_tensor(name, [P, NT], F32))
        w1s = sb("w1s", KD * HH)
        w2s = sb("w2s", KD * HH)
        w3s = sb("w3s", TH * D)
        xTs = [sb(f"xTs{i}", KD * NT) for i in range(2)]
        hTs = [sb(f"hTs{i}", TH * NT) for i in range(2)]
        ss = [sb(f"ss{i}", NT, F32) for i in range(2)]
        sa = [sb(f"sa{i}", NT, F32) for i in range(2)]
        yb = sb("yb", OD * NT, F32)
        yp = sb("yp", OD * NT, F32)
        pa = [ps(f"pa{i}") for i in range(2)]
        pb = [ps(f"pb{i}") for i in range(2)]
        py = [ps(f"py{i}") for i in range(2)]

        sem = lambda name: ctx.enter_context(nc.semaphore(name))
        # Weight loads are split into per-slice DMAs with one sem each:
        # value-gating a single sem across DMAs that complete out of order
        # is unsound (any 16-increment would satisfy the wait).
        s_w1s = [sem(f"s_w1s{j}") for j in range(TH)]
        s_w2s = [sem(f"s_w2s{j}") for j in range(TH)]
        s_w3s = [sem(f"s_w3s{j}") for j in range(OD)]
        s_x = [sem(f"s_x{i}") for i in range(2)]   # +16 per x DMA, by slot parity
        s_x0k = [sem(f"s_x0k{k}") for k in range(KD)]  # +16 per k-tile of x(0)
        s_pa = sem("s_pa")        # +1 per finished pa accumulation group
        s_pb = sem("s_pb")        # +1 per finished pb accumulation group
        s_act = sem("s_act")      # +1 per sigmoid done on ACT (ss written)
        s_sa = sem("s_sa")        # +1 per sa mul done on DVE (ss + pa free)
        s_hT = sem("s_hT")        # +1 per hT tile written by DVE (pb free)
        s_py = sem("s_py")        # +1 per finished py accumulation group
        s_ydone = sem("s_ydone")  # +1 per yb column tile staged by DVE (py free)
        s_yps = [sem(f"s_yps{j}") for j in range(OD)]  # +16 per od reload
        s_store = sem("s_store")  # +16 per y store DMA

        block = ctx.enter_context(nc.Block())

        @block.sync
        def _(sync):
            def w_loads(sync, h):
                # Weight loads for half h, sliced so the half's first block
                # can start after one slice; w1/w2 interleaved to match the
                # pa/pb consumption order. Half-1 reloads gate per slice on
                # the last half-0 reader of that slice (PE completes in
                # order), so they can be issued during half 0's last block.
                hs = h * HH
                w1r = w1s[:].rearrange("p (k c) -> p k c", k=KD)
                w2r = w2s[:].rearrange("p (k c) -> p k c", k=KD)
                for j in range(TH):
                    if h == 1:
                        sync.wait_ge(s_pa, (NB - 1) * TH + j + 1)
                    sync.dma_start(
                        w1r[:, :, j * P:(j + 1) * P],
                        w1_d[:, hs + j * P: hs + (j + 1) * P]
                        .rearrange("(k p) c -> p k c", p=P),
                    ).then_inc(s_w1s[j], 16)
                    if h == 1:
                        sync.wait_ge(s_pb, (NB - 1) * TH + j + 1)
                    sync.dma_start(
                        w2r[:, :, j * P:(j + 1) * P],
                        w2_d[:, hs + j * P: hs + (j + 1) * P]
                        .rearrange("(k p) c -> p k c", p=P),
                    ).then_inc(s_w2s[j], 16)
                    if h == 0 and j == 0:
                        sync.dma_start(
                            xTs[1][:].rearrange("p (k n) -> p k n", k=KD),
                            dram3(xT_d, (1 % NB) * NT),
                        ).then_inc(s_x[1], 16)
                w3r = w3s[:].rearrange("p (k c) -> p k c", k=TH)
                for j in range(OD):
                    if h == 1:
                        sync.wait_ge(s_py, (NB - 1) * OD + j + 1)
                    sync.dma_start(
                        w3r[:, :, j * P:(j + 1) * P],
                        w3_d[hs:hs + HH, j * P:(j + 1) * P]
                        .rearrange("(k p) c -> p k c", p=P),
                    ).then_inc(s_w3s[j], 16)

            for h in range(2):
                if h == 0:
                    # x block 0 split per k-tile so the very first pa group
                    # starts after one 256KB piece instead of the whole 2MB
                    for k in range(KD):
                        sync.dma_start(
                            xTs[0][:, k * NT: k * NT + BS[0]],
                            xT_d[k * P:(k + 1) * P, 0:BS[0]],
                        ).then_inc(s_x0k[k], 16)
                    w_loads(sync, 0)
                if h == 1:
                    # all half-0 partial stores complete before first reload
                    sync.wait_ge(s_store, 16 * OD * NB)
                    for od in range(OD):
                        sync.dma_start(
                            yp[:, od * NT: od * NT + BS[0]],
                            yT_d[od * P:(od + 1) * P, 0:BS[0]],
                        ).then_inc(s_yps[od], 16)
                for b in range(NB):
                    B = h * NB + b
                    # prefetch x two blocks ahead (the store waits below
                    # resolve deep into block B+1, so one-ahead is too late):
                    # slot (B+2)%2 is free once PE's pb groups of block B
                    # completed (pb is the last x reader, in-order)
                    if B + 2 < NBT:
                        j2 = (B + 2) % NB
                        sync.wait_ge(s_pb, TH * (B + 1))
                        sync.dma_start(
                            xTs[B % 2][:]
                            .rearrange("p (k n) -> p k n", k=KD)[:, :, :BS[j2]],
                            dram3(xT_d, j2 * NT, BS[j2]),
                        ).then_inc(s_x[B % 2], 16)
                    if h == 0 and b == NB - 1:
                        # issue half-1 weight reloads before this block's
                        # store waits: their gates resolve during this block,
                        # so the transfers overlap its compute instead of
                        # stalling half 1's first matmuls
                        w_loads(sync, 1)
                    # store each staged od tile of block B as soon as it's
                    # ready (copies of block B land during global block B+1);
                    # in half 1 the od-slice of the next y-partial reload can
                    # go out on the same gate (its slot's reader just ran)
                    for od in range(OD):
                        sync.wait_ge(s_ydone, OD * B + od + 1)
                        if h == 1 and b + 1 < NB:
                            sync.dma_start(
                                yp[:, od * NT: od * NT + BS[b + 1]],
                                yT_d[od * P:(od + 1) * P,
                                     (b + 1) * NT: (b + 1) * NT + BS[b + 1]],
                            ).then_inc(s_yps[od], 16)
                        sync.dma_start(
                            yT_d[od * P:(od + 1) * P, b * NT: b * NT + BS[b]],
                            yb[:, od * NT: od * NT + BS[b]],
                        ).then_inc(s_store, 16)
            sync.wait_ge(s_store, 16 * OD * NBT)

        @block.tensor
        def _(tensor):
            def py_group(Bp, od, chase=False):
                # third matmul for block Bp, interleaved into block Bp+1
                hp, bp = divmod(Bp, NB)
                o = Bp * OD + od
                if od == 0 and not chase:
                    tensor.wait_ge(s_hT, TH * (Bp + 1))    # hT block complete
                if bp == 0:
                    tensor.wait_ge(s_w3s[od], 16 * (hp + 1))
                if o >= 2:
                    tensor.wait_ge(s_ydone, o - 1)         # py slot free
                wp = BS[Bp % NB]
                for k in range(TH):
                    if chase and od == 0:
                        # trailing group right after the block's ht loop:
                        # chase the DVE hT tiles instead of waiting for all
                        tensor.wait_ge(s_hT, Bp * TH + k + 1)
                    mm = tensor.matmul(
                        py[o % 2][:, :wp],
                        w3s[:, k * D + od * P: k * D + od * P + P],
                        hTs[Bp % 2][:, k * NT: k * NT + wp],
                        start=(k == 0), stop=(k == TH - 1),
                    )
                mm.then_inc(s_py, 1)

            for h in range(2):
                for b in range(NB):
                    B = h * NB + b
                    if B == 0:
                        pass          # gated per k-tile inside the first group
                    elif B % 2 == 0:
                        # the x(0) preamble no longer bumps s_x[0]: even
                        # blocks' counts shift down by one
                        tensor.wait_ge(s_x[0], 16 * (B // 2))
                    else:
                        tensor.wait_ge(s_x[1], 16 * (B // 2 + 1))
                    for ht in range(TH):
                        g = B * TH + ht
                        if b == 0:
                            tensor.wait_ge(s_w1s[ht], 16 * (h + 1))
                        if g >= 2:
                            tensor.wait_ge(s_sa, g - 1)    # pa slot free
                        for k in range(KD):
                            if B == 0 and ht == 0:
                                tensor.wait_ge(s_x0k[k], 16)
                            mm = tensor.matmul(
                                pa[g % 2][:, :BS[b]],
                                w1s[:, k * HH + ht * P: k * HH + ht * P + P],
                                xTs[B % 2][:, k * NT: k * NT + BS[b]],
                                start=(k == 0), stop=(k == KD - 1),
                            )
                        mm.then_inc(s_pa, 1)
                        if b == 0:
                            tensor.wait_ge(s_w2s[ht], 16 * (h + 1))
                        if g >= 2:
                            tensor.wait_ge(s_hT, g - 1)    # pb slot free
                        for k in range(KD):
                            mm = tensor.matmul(
                                pb[g % 2][:, :BS[b]],
                                w2s[:, k * HH + ht * P: k * HH + ht * P + P],
                                xTs[B % 2][:, k * NT: k * NT + BS[b]],
                                start=(k == 0), stop=(k == KD - 1),
                            )
                        mm.then_inc(s_pb, 1)
                        if b >= 1:
                            py_group(B - 1, ht)
                # the half's last block can't defer its py groups into the
                # next half: the weight reloads would deadlock against them
                for od in range(OD):
                    py_group(h * NB + NB - 1, od, chase=True)

        @block.scalar
        def _(scalar):
            for g in range(NBT * TH):
                w = BS[(g // TH) % NB]
                scalar.wait_ge(s_pa, g + 1)
                if g >= 2:
                    scalar.wait_ge(s_sa, g - 1)            # ss slot free
                scalar.activation(ss[g % 2][:, :w], pa[g % 2][:, :w],
                                  mybir.ActivationFunctionType.Sigmoid
                                  ).then_inc(s_act, 1)

        @block.vector
        def _(vector):
            def stage_y(Bp, od):
                # stage block Bp's od tile into yb (runs during block Bp+1)
                hp, bp = divmod(Bp, NB)
                o = Bp * OD + od
                if hp == 1:
                    vector.wait_ge(s_yps[od], 16 * (bp + 1))  # reload landed
                if od == 0 and Bp >= 1:
                    vector.wait_ge(s_store, 16 * OD * Bp)      # yb free
                wp = BS[bp]
                vector.wait_ge(s_py, o + 1)
                if hp == 0:
                    vector.tensor_copy(
                        yb[:, od * NT: od * NT + wp],
                        py[o % 2][:, :wp]).then_inc(s_ydone, 1)
                else:
                    vector.tensor_add(
                        yb[:, od * NT: od * NT + wp], py[o % 2][:, :wp],
                        yp[:, od * NT: od * NT + wp],
                    ).then_inc(s_ydone, 1)

            for B in range(NBT):
                if B >= 2:
                    vector.wait_ge(s_py, OD * (B - 1))  # hTs[B%2] free
                w = BS[B % NB]
                for ht in range(TH):
                    g = B * TH + ht
                    vector.wait_ge(s_act, g + 1)
                    vector.tensor_mul(sa[g % 2][:, :w], ss[g % 2][:, :w],
                                      pa[g % 2][:, :w]).then_inc(s_sa, 1)
                    vector.wait_ge(s_sa, g + 1)   # DVE completes async: RAW
                    vector.wait_ge(s_pb, g + 1)
                    vector.tensor_mul(hTs[B % 2][:, ht * NT: ht * NT + w],
                                      sa[g % 2][:, :w], pb[g % 2][:, :w]
                                      ).then_inc(s_hT, 1)
                    if B % NB >= 1:
                        stage_y(B - 1, ht)
                if B % NB == NB - 1:
                    for od in range(OD):
                        stage_y(B, od)
    return nc


def _route(xf: np.ndarray, router_w: np.ndarray):
    """Host router: softmax top-2 with renormalized gates (fp32, matches ref)."""
    logits = xf @ router_w.T                      # [N, E]
    m = logits.max(axis=-1, keepdims=True)
    ex = np.exp(logits - m)
    probs = ex / ex.sum(axis=-1, keepdims=True)
    order = np.argsort(-probs, axis=-1, kind="stable")   # ties: lower idx first
    top2 = order[:, :TOP_K]                       # [N, 2]
    vals = np.take_along_axis(probs, top2, axis=-1)
    gates = vals / vals.sum(axis=-1, keepdims=True)
    return top2, gates.astype(np.float32)


def kernel(x, router_w, w1, w2, w3):
    x = np.asarray(x, np.float32)
    router_w = np.asarray(router_w, np.float32)
    w1 = np.ascontiguousarray(np.asarray(w1, np.float32))
    w2 = np.ascontiguousarray(np.asarray(w2, np.float32))
    w3 = np.ascontiguousarray(np.asarray(w3, np.float32))
    orig_shape = x.shape
    xf = np.ascontiguousarray(x.reshape(-1, D))
    N = xf.shape[0]

    top2, gates = _route(xf, router_w)

    # Dispatch: token index list per expert (token order preserved).
    idx = [np.where((top2 == e).any(axis=1))[0] for e in range(E)]
    gate = [
        np.where(top2[idx[e]] == e, gates[idx[e]], 0.0).sum(axis=1).astype(np.float32)
        for e in range(E)
    ]
    counts = [len(i) for i in idx]
    C = max(((max(counts) + 255) // 256) * 256, 2 * NT)

    if C not in _PROGRAM_CACHE:
        _PROGRAM_CACHE[C] = _build_program(C)
    nc = _PROGRAM_CACHE[C]

    in_maps = []
    for e in range(E):
        xg = np.zeros((C, D), np.float32)
        xg[:counts[e]] = xf[idx[e]]
        in_maps.append({
            "xT": np.ascontiguousarray(xg.T),
            "w1": w1[e],
            "w2": w2[e],
            "w3": w3[e],
        })

    res = run_bass_kernel_spmd(nc, in_maps, core_ids=list(range(E)))
    global LAST_RESULTS
    LAST_RESULTS = res

    out = np.zeros((N, D), np.float32)
    for e in range(E):
        y = res.results[e]["yT"][:, :counts[e]].T          # [count, D]
        out[idx[e]] += gate[e][:, None] * y
    return out.reshape(orig_shape), np.float32(0.0)


# revision 53
# speedup vs baseline: 1.0662x; 1.0055x over previous
"""MoE SwiGLU (top-2 of 8 experts) on 8 TRN2 NeuronCores.

Strategy: expert-parallel. The router (tiny: [N,1024]@[1024,8]) plus the
top-2 dispatch/combine permutations run on host as part of the sharding
step -- sharding by expert is only possible after routing, and the
all-to-all dispatch/combine of the sharding hint is exactly this
host-side gather/scatter under the full-I/O contract. Each core then
runs a dense SwiGLU FFN over its expert's gathered tokens (padded to a
fixed capacity C):

    yT = w3.T @ (silu(w1.T @ xT) * (w2.T @ xT))      all on-device

Everything is kept transposed ([feature, token]) so no on-device
transposes are needed: w1/w2 ([D,H]) and w3 ([H,D]) are already in lhsT
layout, x is shipped pre-transposed, y returns transposed.

Matmuls run as float32r (full fp32 data, 1 cycle/row on the PE at free
dim >= 256, vs 4 cycles/row for plain fp32). The hidden dim (2048) is
processed in two halves so the resident weight set fits SBUF; each
half's partial y is accumulated through the output DRAM buffer.

The device program is raw Bass (explicit per-engine streams and
semaphores, no TileContext): the walrus build in this container accepts
at most ONE semaphore wait per instruction, which Tile's auto-generated
sync violates structurally (slot-recycling WAR/WAW sets span multiple
procs). With explicit streams, every wait is its own single-wait
wait_ge instruction and every DMA is enqueued only after its
dependencies completed, so no instruction ever carries two waits.
"""

import numpy as np

import concourse.bass as bass
import concourse.mybir as mybir
from concourse.bass_utils import run_bass_kernel_spmd

D = 1024          # model dim
H = 2048          # expert hidden dim
HH = H // 2       # hidden half processed per pass (weight residency)
E = 8             # experts == cores
TOP_K = 2
NT = 512          # tokens per block (max fp32 moving free dim)
P = 128

F32 = mybir.dt.float32
F32R = mybir.dt.float32r

KD = D // P       # k-tiles over model dim (contraction of matmul 1/2)
TH = HH // P      # hidden tiles per half (contraction of matmul 3)
OD = D // P       # output dim tiles

_PROGRAM_CACHE: dict[int, bass.Bass] = {}
LAST_RESULTS = None  # BassKernelResults of the most recent run (for test harness)


def _build_program(C: int) -> bass.Bass:
    """One core's program: dense SwiGLU FFN over C tokens, transposed layout."""
    BS = [NT] * (C // NT) + ([C % NT] if C % NT else [])   # per-block widths
    assert C % NT in (0, 256) and len(BS) >= 2
    NB = len(BS)
    NBT = 2 * NB            # total block passes (two hidden halves)
    nc = bass.Bass("TRN2", target_bir_lowering=False, debug=False)
    xT_d = nc.dram_tensor("xT", [D, C], F32R, kind="ExternalInput").ap()
    w1_d = nc.dram_tensor("w1", [D, H], F32R, kind="ExternalInput").ap()
    w2_d = nc.dram_tensor("w2", [D, H], F32R, kind="ExternalInput").ap()
    w3_d = nc.dram_tensor("w3", [H, D], F32R, kind="ExternalInput").ap()
    yT_d = nc.dram_tensor("yT", [D, C], F32, kind="ExternalOutput").ap()

    def dram3(t, c0, w=NT):
        # [D, w] column block as [p, k, n] for a single strided DMA
        return t[:, c0:c0 + w].rearrange("(k p) n -> p k n", p=P)

    from contextlib import ExitStack
    with ExitStack() as ctx:
        sb = lambda name, cols, dt=F32R: ctx.enter_context(
            nc.sbuf_tensor(name, [P, cols], dt))
        ps = lambda name: ctx.enter_context(
            nc.psum_tensor(name, [P, NT], F32))
        w1s = sb("w1s", KD * HH)
        w2s = sb("w2s", KD * HH)
        w3s = sb("w3s", TH * D)
        xTs = [sb(f"xTs{i}", KD * NT) for i in range(2)]
        hTs = [sb(f"hTs{i}", TH * NT) for i in range(2)]
        ss = [sb(f"ss{i}", NT, F32) for i in range(2)]
        sa = [sb(f"sa{i}", NT, F32) for i in range(2)]
        yb = sb("yb", OD * NT, F32)
        yp = sb("yp", OD * NT, F32)
        pa = [ps(f"pa{i}") for i in range(3)]
        pb = [ps(f"pb{i}") for i in range(2)]
        py = [ps(f"py{i}") for i in range(3)]   # pa3+pb2+py3 = 8 PSUM banks

        sem = lambda name: ctx.enter_context(nc.semaphore(name))
        # Weight loads are split into per-slice DMAs with one sem each:
        # value-gating a single sem across DMAs that complete out of order
        # is unsound (any 16-increment would satisfy the wait).
        s_w1s = [sem(f"s_w1s{j}") for j in range(TH)]
        s_w2s = [sem(f"s_w2s{j}") for j in range(TH)]
        s_w3s = [sem(f"s_w3s{j}") for j in range(OD)]
        s_x = [sem(f"s_x{i}") for i in range(2)]   # +16 per x DMA, by slot parity
        s_x0k = [sem(f"s_x0k{k}") for k in range(KD)]  # +16 per k-tile of x(0)
        s_pa = sem("s_pa")        # +1 per finished pa accumulation group
        s_pb = sem("s_pb")        # +1 per finished pb accumulation group
        s_act = sem("s_act")      # +1 per sigmoid done on ACT (ss written)
        s_sa = sem("s_sa")        # +1 per sa mul done on DVE (ss + pa free)
        s_hT = sem("s_hT")        # +1 per hT tile written by DVE (pb free)
        s_py = sem("s_py")        # +1 per finished py accumulation group
        s_ydone = sem("s_ydone")  # +1 per yb column tile staged by DVE (py free)
        s_yps = [sem(f"s_yps{j}") for j in range(OD)]  # +16 per od reload
        s_store = sem("s_store")  # +16 per y store DMA

        block = ctx.enter_context(nc.Block())

        @block.sync
        def _(sync):
            def w_loads(sync, h):
                # Weight loads for half h, sliced so the half's first block
                # can start after one slice; w1/w2 interleaved to match the
                # pa/pb consumption order. Half-1 reloads gate per slice on
                # the last half-0 reader of that slice (PE completes in
                # order), so they can be issued during half 0's last block.
                hs = h * HH
                w1r = w1s[:].rearrange("p (k c) -> p k c", k=KD)
                w2r = w2s[:].rearrange("p (k c) -> p k c", k=KD)
                for j in range(TH):
                    if h == 1:
                        sync.wait_ge(s_pa, (NB - 1) * TH + j + 1)
                    sync.dma_start(
                        w1r[:, :, j * P:(j + 1) * P],
                        w1_d[:, hs + j * P: hs + (j + 1) * P]
                        .rearrange("(k p) c -> p k c", p=P),
                    ).then_inc(s_w1s[j], 16)
                    if h == 1:
                        sync.wait_ge(s_pb, (NB - 1) * TH + j + 1)
                    sync.dma_start(
                        w2r[:, :, j * P:(j + 1) * P],
                        w2_d[:, hs + j * P: hs + (j + 1) * P]
                        .rearrange("(k p) c -> p k c", p=P),
                    ).then_inc(s_w2s[j], 16)
                    if h == 0 and j == 0:
                        sync.dma_start(
                            xTs[1][:].rearrange("p (k n) -> p k n", k=KD),
                            dram3(xT_d, (1 % NB) * NT),
                        ).then_inc(s_x[1], 16)
                w3r = w3s[:].rearrange("p (k c) -> p k c", k=TH)
                for j in range(OD):
                    if h == 1:
                        sync.wait_ge(s_py, (NB - 1) * OD + j + 1)
                    sync.dma_start(
                        w3r[:, :, j * P:(j + 1) * P],
                        w3_d[hs:hs + HH, j * P:(j + 1) * P]
                        .rearrange("(k p) c -> p k c", p=P),
                    ).then_inc(s_w3s[j], 16)

            for h in range(2):
                if h == 0:
                    # x block 0 split per k-tile so the very first pa group
                    # starts after one 256KB piece instead of the whole 2MB
                    for k in range(KD):
                        sync.dma_start(
                            xTs[0][:, k * NT: k * NT + BS[0]],
                            xT_d[k * P:(k + 1) * P, 0:BS[0]],
                        ).then_inc(s_x0k[k], 16)
                    w_loads(sync, 0)
                if h == 1:
                    # all half-0 partial stores complete before first reload
                    sync.wait_ge(s_store, 16 * OD * NB)
                    for od in range(OD):
                        sync.dma_start(
                            yp[:, od * NT: od * NT + BS[0]],
                            yT_d[od * P:(od + 1) * P, 0:BS[0]],
                        ).then_inc(s_yps[od], 16)
                for b in range(NB):
                    B = h * NB + b
                    # prefetch x two blocks ahead (the store waits below
                    # resolve deep into block B+1, so one-ahead is too late):
                    # slot (B+2)%2 is free once PE's pb groups of block B
                    # completed (pb is the last x reader, in-order)
                    if B + 2 < NBT:
                        j2 = (B + 2) % NB
                        sync.wait_ge(s_pb, TH * (B + 1))
                        sync.dma_start(
                            xTs[B % 2][:]
                            .rearrange("p (k n) -> p k n", k=KD)[:, :, :BS[j2]],
                            dram3(xT_d, j2 * NT, BS[j2]),
                        ).then_inc(s_x[B % 2], 16)
                    if h == 0 and b == NB - 1:
                        # issue half-1 weight reloads before this block's
                        # store waits: their gates resolve during this block,
                        # so the transfers overlap its compute instead of
                        # stalling half 1's first matmuls
                        w_loads(sync, 1)
                    # store each staged od tile of block B as soon as it's
                    # ready (copies of block B land during global block B+1);
                    # in half 1 the od-slice of the next y-partial reload can
                    # go out on the same gate (its slot's reader just ran)
                    for od in range(OD):
                        sync.wait_ge(s_ydone, OD * B + od + 1)
                        if h == 1 and b + 1 < NB:
                            sync.dma_start(
                                yp[:, od * NT: od * NT + BS[b + 1]],
                                yT_d[od * P:(od + 1) * P,
                                     (b + 1) * NT: (b + 1) * NT + BS[b + 1]],
                            ).then_inc(s_yps[od], 16)
                        sync.dma_start(
                            yT_d[od * P:(od + 1) * P, b * NT: b * NT + BS[b]],
                            yb[:, od * NT: od * NT + BS[b]],
                        ).then_inc(s_store, 16)
            sync.wait_ge(s_store, 16 * OD * NBT)

        @block.tensor
        def _(tensor):
            def py_group(Bp, od, chase=False):
                # third matmul for block Bp, interleaved into block Bp+1
                hp, bp = divmod(Bp, NB)
                o = Bp * OD + od
                if od == 0 and not chase:
                    tensor.wait_ge(s_hT, TH * (Bp + 1))    # hT block complete
                if bp == 0:
                    tensor.wait_ge(s_w3s[od], 16 * (hp + 1))
                if o >= 3:
                    tensor.wait_ge(s_ydone, o - 2)         # py slot free
                wp = BS[Bp % NB]
                for k in range(TH):
                    if chase and od == 0:
                        # trailing group right after the block's ht loop:
                        # chase the DVE hT tiles instead of waiting for all
                        tensor.wait_ge(s_hT, Bp * TH + k + 1)
                    mm = tensor.matmul(
                        py[o % 3][:, :wp],
                        w3s[:, k * D + od * P: k * D + od * P + P],
                        hTs[Bp % 2][:, k * NT: k * NT + wp],
                        start=(k == 0), stop=(k == TH - 1),
                    )
                mm.then_inc(s_py, 1)

            for h in range(2):
                for b in range(NB):
                    B = h * NB + b
                    if B == 0:
                        pass          # gated per k-tile inside the first group
                    elif B % 2 == 0:
                        # the x(0) preamble no longer bumps s_x[0]: even
                        # blocks' counts shift down by one
                        tensor.wait_ge(s_x[0], 16 * (B // 2))
                    else:
                        tensor.wait_ge(s_x[1], 16 * (B // 2 + 1))
                    for ht in range(TH):
                        g = B * TH + ht
                        if b == 0:
                            tensor.wait_ge(s_w1s[ht], 16 * (h + 1))
                        if g >= 3:
                            tensor.wait_ge(s_sa, g - 2)    # pa slot free
                        for k in range(KD):
                            if B == 0 and ht == 0:
                                tensor.wait_ge(s_x0k[k], 16)
                            mm = tensor.matmul(
                                pa[g % 3][:, :BS[b]],
                                w1s[:, k * HH + ht * P: k * HH + ht * P + P],
                                xTs[B % 2][:, k * NT: k * NT + BS[b]],
                                start=(k == 0), stop=(k == KD - 1),
                            )
                        mm.then_inc(s_pa, 1)
                        if b == 0:
                            tensor.wait_ge(s_w2s[ht], 16 * (h + 1))
                        if g >= 2:
                            tensor.wait_ge(s_hT, g - 1)    # pb slot free
                        for k in range(KD):
                            mm = tensor.matmul(
                                pb[g % 2][:, :BS[b]],
                                w2s[:, k * HH + ht * P: k * HH + ht * P + P],
                                xTs[B % 2][:, k * NT: k * NT + BS[b]],
                                start=(k == 0), stop=(k == KD - 1),
                            )
                        mm.then_inc(s_pb, 1)
                        if b >= 1:
                            py_group(B - 1, ht)
                # the half's last block can't defer its py groups into the
                # next half: the weight reloads would deadlock against them
                for od in range(OD):
                    py_group(h * NB + NB - 1, od, chase=True)

        @block.scalar
        def _(scalar):
            for g in range(NBT * TH):
                w = BS[(g // TH) % NB]
                scalar.wait_ge(s_pa, g + 1)
                if g >= 2:
                    scalar.wait_ge(s_sa, g - 1)            # ss slot free
                scalar.activation(ss[g % 2][:, :w], pa[g % 3][:, :w],
                                  mybir.ActivationFunctionType.Sigmoid
                                  ).then_inc(s_act, 1)

        @block.vector
        def _(vector):
            def stage_y(Bp, od):
                # stage block Bp's od tile into yb (runs during block Bp+1)
                hp, bp = divmod(Bp, NB)
                o = Bp * OD + od
                if hp == 1:
                    vector.wait_ge(s_yps[od], 16 * (bp + 1))  # reload landed
                if od == 0 and Bp >= 1:
                    vector.wait_ge(s_store, 16 * OD * Bp)      # yb free
                wp = BS[bp]
                vector.wait_ge(s_py, o + 1)
                if hp == 0:
                    vector.tensor_copy(
                        yb[:, od * NT: od * NT + wp],
                        py[o % 3][:, :wp]).then_inc(s_ydone, 1)
                else:
                    vector.tensor_add(
                        yb[:, od * NT: od * NT + wp], py[o % 3][:, :wp],
                        yp[:, od * NT: od * NT + wp],
                    ).then_inc(s_ydone, 1)

            for B in range(NBT):
                if B >= 2:
                    vector.wait_ge(s_py, OD * (B - 1))  # hTs[B%2] free
                w = BS[B % NB]
                for ht in range(TH):
                    g = B * TH + ht
                    vector.wait_ge(s_act, g + 1)
                    vector.tensor_mul(sa[g % 2][:, :w], ss[g % 2][:, :w],
                                      pa[g % 3][:, :w]).then_inc(s_sa, 1)
                    vector.wait_ge(s_sa, g + 1)   # DVE completes async: RAW
                    vector.wait_ge(s_pb, g + 1)
                    vector.tensor_mul(hTs[B % 2][:, ht * NT: ht * NT + w],
                                      sa[g % 2][:, :w], pb[g % 2][:, :w]
                                      ).then_inc(s_hT, 1)
                    if B % NB >= 1:
                        stage_y(B - 1, ht)
                if B % NB == NB - 1:
                    for od in range(OD):
                        stage_y(B, od)
    return nc


def _route(xf: np.ndarray, router_w: np.ndarray):
    """Host router: softmax top-2 with renormalized gates (fp32, matches ref)."""
    logits = xf @ router_w.T                      # [N, E]
    m = logits.max(axis=-1, keepdims=True)
    ex = np.exp(logits - m)
    probs = ex / ex.sum(axis=-1, keepdims=True)
    order = np.argsort(-probs, axis=-1, kind="stable")   # ties: lower idx first
    top2 = order[:, :TOP_K]                       # [N, 2]
    vals = np.take_along_axis(probs, top2, axis=-1)
    gates = vals / vals.sum(axis=-1, keepdims=True)
    return top2, gates.astype(np.float32)


def kernel(x, router_w, w1, w2, w3):
    x = np.asarray(x, np.float32)
    router_w = np.asarray(router_w, np.float32)
    w1 = np.ascontiguousarray(np.asarray(w1, np.float32))
    w2 = np.ascontiguousarray(np.asarray(w2, np.float32))
    w3 = np.ascontiguousarray(np.asarray(w3, np.float32))
    orig_shape = x.shape
    xf = np.ascontiguousarray(x.reshape(-1, D))
    N = xf.shape[0]

    top2, gates = _route(xf, router_w)

    # Dispatch: token index list per expert (token order preserved).
    idx = [np.where((top2 == e).any(axis=1))[0] for e in range(E)]
    gate = [
        np.where(top2[idx[e]] == e, gates[idx[e]], 0.0).sum(axis=1).astype(np.float32)
        for e in range(E)
    ]
    counts = [len(i) for i in idx]
    C = max(((max(counts) + 255) // 256) * 256, 2 * NT)

    if C not in _PROGRAM_CACHE:
        _PROGRAM_CACHE[C] = _build_program(C)
    nc = _PROGRAM_CACHE[C]

    in_maps = []
    for e in range(E):
        xg = np.zeros((C, D), np.float32)
        xg[:counts[e]] = xf[idx[e]]
        in_maps.append({
            "xT": np.ascontiguousarray(xg.T),
            "w1": w1[e],
            "w2": w2[e],
            "w3": w3[e],
        })

    res = run_bass_kernel_spmd(nc, in_maps, core_ids=list(range(E)))
    global LAST_RESULTS
    LAST_RESULTS = res

    out = np.zeros((N, D), np.float32)
    for e in range(E):
        y = res.results[e]["yT"][:, :counts[e]].T          # [count, D]
        out[idx[e]] += gate[e][:, None] * y
    return out.reshape(orig_shape), np.float32(0.0)
